# revision 8
# baseline (speedup 1.0000x reference)
"""DCT sequence-compression kernel for TRN2 (nn_CompressedModel).

For x [B=64, T=1024, D=768] fp32 computes (matching the reference):
  x_dct = (C_T @ x)[:, :k, :]          k = 922
  x_rec = C_k^T @ x_dct
returning (x_rec, x_dct).

Structure exploited (each identity verified to ~1e-13 vs the reference):
 1) Input mirror folds (host-side data prep, O(B T D) adds):
      e = x[:512] + rev(x[512:]),  o = x[:512] - rev(x[512:])
      ee = e[:256] + rev(e[256:]), eo = e[:256] - rev(e[256:])
    dct rows 4i contract only ee (256), rows 4i+2 only eo (256),
    odd rows only o (512).
 2) Output-row fold of the reconstruction: with W2 = C_T[:K].T @ C_k,
    W2[t, 921-n] = W2[1023-t, n]. Folding twice:
      v = A.T o            (A = asym half of W2, 512 x 461)
      p = Sp.T ee          (Sp from sym half S, 256 x 231)
      q = Sm.T eo          (Sm,                 256 x 230)
      u[j] = (p+q)/2, u[460-j] = (p-q)/2,
      rec[n] = u[n] + v[n], rec[921-n] = u[n] - v[n].
    The u/rec recombination is O(B K D) elementwise host work; the
    device emits the six matmul group outputs (d_ee, d_eo, d_o, p, q,
    v = 1844 rows) packed in one DRAM tensor.

Tensor-engine streaming per batch: (2*2 + 2*2 + 4*4 + 2*2 + 2*2 + 4*4)
units x 768 free-dim cycles = 36864 cycles, vs 73728 for the original
kernel. Matmuls run in bf16 by default (~3e-3 rel err, FWL fast weight
loads) or float32r (KERNEL_DTYPE=f32r, ~1.5e-4). Pure data parallel
over B across 8 cores.
"""

import os

import numpy as np

# The trimmed axon environment has no NTFF profile hook; make sure
# run_bass_kernel_spmd never tries the trace path.
os.environ["BASS_NEVER_TRACE"] = "1"

import concourse.bass as bass  # noqa: F401
import concourse.mybir as mybir
import concourse.tile as tile
from concourse import bacc
from concourse.bass_utils import run_bass_kernel_spmd

B, T, D = 64, 1024, 768
K = 922              # ceil(0.9 * 1024)
H = T // 2           # 512: o contraction length
Q = T // 4           # 256: ee/eo contraction length
NU = K // 2          # 461: dct odd rows, v rows
NEE = 231            # dct rows 4i (k <= 920), p rows
NEO = 230            # dct rows 4i+2 (k <= 918), q rows
N_CORES = 8
BPC = B // N_CORES   # batches per core
P = 128
CCO = H // P         # 4 contraction chunks for o
CCE = Q // P         # 2 contraction chunks for ee/eo
N0 = 512             # PSUM-bank split of the free dim

# packed output row regions: d_ee | d_eo | d_o | p | q | v
R_DEE, R_DEO, R_DO = 0, NEE, NEE + NEO
R_P, R_Q, R_V = R_DO + NU, R_DO + NU + NEE, R_DO + NU + NEE + NEO
R_TOT = R_V + NU     # 1844

DTYPE_NAME = os.environ.get("KERNEL_DTYPE", "bf16")
if DTYPE_NAME == "bf16":
    import ml_dtypes
    MM_DTYPE = mybir.dt.bfloat16
    NP_IN = ml_dtypes.bfloat16
    OUT_DTYPE = mybir.dt.bfloat16
else:
    MM_DTYPE = mybir.dt.float32r
    NP_IN = np.float32
    OUT_DTYPE = mybir.dt.float32


def _chunks(n, p=P):
    return [(i * p, min(p, n - i * p)) for i in range((n + p - 1) // p)]


EE_CHUNKS = _chunks(NEE)   # [(0,128),(128,103)]
EO_CHUNKS = _chunks(NEO)   # [(0,128),(128,102)]
O_CHUNKS = _chunks(NU)     # 4 chunks


def _dct_matrix(N: int) -> np.ndarray:
    """Orthonormal DCT-II matrix [N, N] in float64."""
    n = np.arange(N, dtype=np.float64)
    C = np.cos(np.pi * (2.0 * n[None, :] + 1.0) * n[:, None] / (2.0 * N))
    s = np.full(N, np.sqrt(2.0 / N))
    s[0] = np.sqrt(1.0 / N)
    return s[:, None] * C


def _build_weights():
    C_T = _dct_matrix(T)
    C_k = _dct_matrix(K)
    W2 = (C_k.T @ C_T[:K, :]).T            # [T, K]
    W2r = W2[::-1, :]
    S = ((W2[:H, :] + W2r[:H, :]) / 2.0)[:, :NU]   # [512, 461]
    A = ((W2[:H, :] - W2r[:H, :]) / 2.0)[:, :NU]   # [512, 461]
    Sp = (S[:Q, :] + S[:Q, ::-1])[:, :NEE]         # [256, 231]
    Sm = (S[:Q, :] - S[:Q, ::-1])[:, :NEO]         # [256, 230]
    wee = np.concatenate([C_T[0:K:4, :Q].T, Sp], axis=1)   # [256, 462]
    weo = np.concatenate([C_T[2:K:4, :Q].T, Sm], axis=1)   # [256, 460]
    wo = np.concatenate([C_T[1:K:2, :H].T, A], axis=1)     # [512, 922]
    return wee.astype(NP_IN), weo.astype(NP_IN), wo.astype(NP_IN)


def _build_bass(loop_repeat: int = 1):
    """loop_repeat>1 wraps the program in a hardware For_i loop (same
    outputs each trip) - used by test.py for slope-based HW timing."""
    f32 = mybir.dt.float32
    nc = bacc.Bacc("TRN2", target_bir_lowering=False, debug=False,
                   num_devices=N_CORES)
    ee_in = nc.dram_tensor("ee", [BPC, Q, D], MM_DTYPE,
                           kind="ExternalInput").ap()
    eo_in = nc.dram_tensor("eo", [BPC, Q, D], MM_DTYPE,
                           kind="ExternalInput").ap()
    o_in = nc.dram_tensor("o", [BPC, H, D], MM_DTYPE,
                          kind="ExternalInput").ap()
    wee_in = nc.dram_tensor("wee", [Q, NEE + NEE], MM_DTYPE,
                            kind="ExternalInput").ap()
    weo_in = nc.dram_tensor("weo", [Q, NEO + NEO], MM_DTYPE,
                            kind="ExternalInput").ap()
    wo_in = nc.dram_tensor("wo", [H, NU + NU], MM_DTYPE,
                           kind="ExternalInput").ap()
    out = nc.dram_tensor("out", [BPC, R_TOT, D], OUT_DTYPE,
                         kind="ExternalOutput").ap()

    # batch-pair staging: [pair, p, 2, cc, d]
    ee_r = ee_in.rearrange("(h two) (c p) d -> h p two c d", two=2, p=P)
    eo_r = eo_in.rearrange("(h two) (c p) d -> h p two c d", two=2, p=P)
    o_r = o_in.rearrange("(h two) (c p) d -> h p two c d", two=2, p=P)
    wee_r = wee_in.rearrange("(c p) j -> p c j", p=P)
    weo_r = weo_in.rearrange("(c p) j -> p c j", p=P)
    wo_r = wo_in.rearrange("(c p) j -> p c j", p=P)

    with tile.TileContext(nc) as tc:
        with (
            tc.tile_pool(name="wp", bufs=1) as wp,
            tc.tile_pool(name="xp", bufs=3) as xp,
            tc.tile_pool(name="op", bufs=8) as op,
            tc.tile_pool(name="pp", bufs=4, space="PSUM") as pp,
        ):
            weet = wp.tile([P, CCE, NEE + NEE], MM_DTYPE)
            weot = wp.tile([P, CCE, NEO + NEO], MM_DTYPE)
            wot = wp.tile([P, CCO, NU + NU], MM_DTYPE)
            # weights stream on the ACT HWDGE ring in consumption order
            for (c0, sz) in EE_CHUNKS:
                nc.scalar.dma_start(weet[:, :, c0:c0 + sz],
                                    wee_r[:, :, c0:c0 + sz])
            for (c0, sz) in EO_CHUNKS:
                nc.scalar.dma_start(weot[:, :, c0:c0 + sz],
                                    weo_r[:, :, c0:c0 + sz])
            for (c0, sz) in O_CHUNKS:
                nc.scalar.dma_start(wot[:, :, c0:c0 + sz],
                                    wo_r[:, :, c0:c0 + sz])
            for (c0, sz) in EE_CHUNKS:
                nc.scalar.dma_start(weet[:, :, NEE + c0:NEE + c0 + sz],
                                    wee_r[:, :, NEE + c0:NEE + c0 + sz])
            for (c0, sz) in EO_CHUNKS:
                nc.scalar.dma_start(weot[:, :, NEO + c0:NEO + c0 + sz],
                                    weo_r[:, :, NEO + c0:NEO + c0 + sz])
            for (c0, sz) in O_CHUNKS:
                nc.scalar.dma_start(wot[:, :, NU + c0:NU + c0 + sz],
                                    wo_r[:, :, NU + c0:NU + c0 + sz])

            def mm_group_pair(pt0, pt1, wtile, ncc, c0, rhs, sz):
                # 4 matmuls per cc share one weight tile; the dedup pass
                # below collapses their Ldweights into one.
                for cc in range(ncc):
                    st, sp = (cc == 0), (cc == ncc - 1)
                    for bi, pt in ((0, pt0), (1, pt1)):
                        nc.tensor.matmul(
                            pt[:sz, 0:N0], wtile[:, cc, c0:c0 + sz],
                            rhs[:, bi, cc, 0:N0], start=st, stop=sp)
                        nc.tensor.matmul(
                            pt[:sz, N0:D], wtile[:, cc, c0:c0 + sz],
                            rhs[:, bi, cc, N0:D], start=st, stop=sp)

            def body():
                for h in range(BPC // 2):
                    b = 2 * h
                    eet = xp.tile([P, 2, CCE, D], MM_DTYPE, tag="eet")
                    eot = xp.tile([P, 2, CCE, D], MM_DTYPE, tag="eot")
                    ot = xp.tile([P, 2, CCO, D], MM_DTYPE, tag="ot")
                    nc.sync.dma_start(eet[:], ee_r[h])
                    nc.sync.dma_start(eot[:], eo_r[h])
                    nc.sync.dma_start(ot[:], o_r[h])

                    units = []
                    for (r0, sz) in EE_CHUNKS:     # dct rows 4i
                        units.append((weet, CCE, r0, eet, R_DEE + r0, sz))
                    for (r0, sz) in EO_CHUNKS:     # dct rows 4i+2
                        units.append((weot, CCE, r0, eot, R_DEO + r0, sz))
                    for (r0, sz) in O_CHUNKS:      # dct odd rows
                        units.append((wot, CCO, r0, ot, R_DO + r0, sz))
                    for (r0, sz) in EE_CHUNKS:     # p
                        units.append((weet, CCE, NEE + r0, eet, R_P + r0, sz))
                    for (r0, sz) in EO_CHUNKS:     # q
                        units.append((weot, CCE, NEO + r0, eot, R_Q + r0, sz))
                    for (r0, sz) in O_CHUNKS:      # v
                        units.append((wot, CCO, NU + r0, ot, R_V + r0, sz))

                    for i, (wtile, ncc, c0, rhs, dr0, sz) in enumerate(units):
                        pt0 = pp.tile([P, D], f32, tag="pt")
                        pt1 = pp.tile([P, D], f32, tag="pt")
                        mm_group_pair(pt0, pt1, wtile, ncc, c0, rhs, sz)
                        for bi, pt in ((0, pt0), (1, pt1)):
                            so = op.tile([P, D], OUT_DTYPE, tag="so")
                            if (2 * i + bi) % 2 == 0:
                                nc.vector.tensor_copy(so[:sz, :], pt[:sz, :])
                            else:
                                nc.scalar.copy(so[:sz, :], pt[:sz, :])
                            nc.sync.dma_start(out[b + bi, dr0:dr0 + sz, :],
                                              so[:sz, :])

            if loop_repeat > 1:
                with tc.For_i(0, loop_repeat, 1):
                    body()
            else:
                body()
    _dedup_ldweights(nc)
    nc.compile()
    return nc


def _dedup_ldweights(nc):
    """Drop an InstLdweights whose weights AP is identical to the previous
    Ldweights on the PE queue with only Matmult/EventSemaphore between -
    the array already holds those weights. The tile legalizer emits one
    Ldweights per matmul unconditionally; with the batch-pair loop 4
    matmuls share each weight tile, so this removes 3/4 of the (~106 ns
    each) PE-sequencer weight-load slots."""
    pe = mybir.EngineType.PE
    for blk in nc.m.functions[0].blocks:
        insts = list(blk.instructions)
        keep = []
        removed = False
        last_key = None
        for inst in insts:
            if getattr(inst, "engine", None) == pe:
                n = type(inst).__name__
                if n == "InstLdweights":
                    key = str(inst.ins[0])
                    if key == last_key:
                        removed = True
                        continue
                    last_key = key
                elif n not in ("InstMatmult", "InstEventSemaphore"):
                    last_key = None
            keep.append(inst)
        if removed:
            blk.instructions = keep


_CACHE = {}


def _get():
    if "nc" not in _CACHE:
        _CACHE["nc"] = _build_bass()
        _CACHE["w"] = _build_weights()
    return _CACHE["nc"], _CACHE["w"]


def _make_in_maps(x: np.ndarray):
    _, w = _get()
    wee, weo, wo = w
    x = np.ascontiguousarray(x, dtype=np.float32)
    lo = x[:, :H, :]
    hi = x[:, :H - 1:-1, :]
    e = lo + hi
    o = np.ascontiguousarray(lo - hi, dtype=NP_IN)
    ee = np.ascontiguousarray(e[:, :Q, :] + e[:, :Q - 1:-1, :], dtype=NP_IN)
    eo = np.ascontiguousarray(e[:, :Q, :] - e[:, :Q - 1:-1, :], dtype=NP_IN)
    return [
        {"ee": ee[c * BPC:(c + 1) * BPC], "eo": eo[c * BPC:(c + 1) * BPC],
         "o": o[c * BPC:(c + 1) * BPC], "wee": wee, "weo": weo, "wo": wo}
        for c in range(N_CORES)
    ]


def kernel(x: np.ndarray, _results_out=None):
    """x [64, 1024, 768] fp32 -> (x_rec [64, 922, 768], x_dct [64, 922, 768])."""
    nc, _ = _get()
    in_maps = _make_in_maps(x)
    res = run_bass_kernel_spmd(nc, in_maps, core_ids=list(range(N_CORES)))
    if _results_out is not None:
        _results_out.append(res)
    out = np.concatenate([r["out"] for r in res.results], axis=0)
    out = out.astype(np.float32)

    d_ee = out[:, R_DEE:R_DEE + NEE]
    d_eo = out[:, R_DEO:R_DEO + NEO]
    d_o = out[:, R_DO:R_DO + NU]
    pp_ = out[:, R_P:R_P + NEE]
    qq = out[:, R_Q:R_Q + NEO]
    vv = out[:, R_V:R_V + NU]

    x_dct = np.empty((B, K, D), dtype=np.float32)
    x_dct[:, 0::4] = d_ee
    x_dct[:, 2::4] = d_eo
    x_dct[:, 1::2] = d_o

    u = np.empty((B, NU, D), dtype=np.float32)
    u[:, :NEO] = (pp_[:, :NEO] + qq) * 0.5
    u[:, NEO] = pp_[:, NEO] * 0.5
    u[:, NEE:] = ((pp_[:, :NEO] - qq) * 0.5)[:, ::-1]
    x_rec = np.concatenate([u + vv, (u - vv)[:, ::-1, :]], axis=1)
    return np.ascontiguousarray(x_rec), np.ascontiguousarray(x_dct)


# revision 10
# speedup vs baseline: 1.1585x; 1.1585x over previous
"""DCT sequence-compression kernel for TRN2 (nn_CompressedModel).

For x [B=64, T=1024, D=768] fp32 computes (matching the reference):
  x_dct = (C_T @ x)[:, :k, :]          k = 922
  x_rec = C_k^T @ x_dct
returning (x_rec, x_dct).

Structure exploited (each identity verified to ~1e-13 vs the reference):
 1) Input mirror folds (host-side data prep, O(B T D) adds):
      e = x[:512] + rev(x[512:]),  o = x[:512] - rev(x[512:])
      ee = e[:256] + rev(e[256:]), eo = e[:256] - rev(e[256:])
    dct rows 4i contract only ee (256), rows 4i+2 only eo (256),
    odd rows only o (512).
 2) Output-row fold of the reconstruction: with W2 = C_T[:K].T @ C_k,
    W2[t, 921-n] = W2[1023-t, n]. Folding twice:
      v = A.T o            (A = asym half of W2, 512 x 461)
      p = Sp.T ee          (Sp from sym half S, 256 x 231)
      q = Sm.T eo          (Sm,                 256 x 230)
      u[j] = (p+q)/2, u[460-j] = (p-q)/2,
      rec[n] = u[n] + v[n], rec[921-n] = u[n] - v[n].
    The u/rec recombination is O(B K D) elementwise host work; the
    device emits the six matmul group outputs (d_ee, d_eo, d_o, p, q,
    v = 1844 rows) packed in one DRAM tensor.

Tensor-engine streaming per batch: (2*2 + 2*2 + 4*4 + 2*2 + 2*2 + 4*4)
units x 768 free-dim cycles = 36864 cycles, vs 73728 for the original
kernel. Matmuls run in bf16 by default (~3e-3 rel err, FWL fast weight
loads) or float32r (KERNEL_DTYPE=f32r, ~1.5e-4). Pure data parallel
over B across 8 cores.
"""

import os

import numpy as np

# The trimmed axon environment has no NTFF profile hook; make sure
# run_bass_kernel_spmd never tries the trace path.
os.environ["BASS_NEVER_TRACE"] = "1"

import concourse.bass as bass  # noqa: F401
import concourse.mybir as mybir
import concourse.tile as tile
from concourse import bacc
from concourse.bass_utils import run_bass_kernel_spmd

B, T, D = 64, 1024, 768
K = 922              # ceil(0.9 * 1024)
H = T // 2           # 512: o contraction length
Q = T // 4           # 256: ee/eo contraction length
NU = K // 2          # 461: dct odd rows, v rows
NEE = 231            # dct rows 4i (k <= 920), p rows
NEO = 230            # dct rows 4i+2 (k <= 918), q rows
N_CORES = 8
BPC = B // N_CORES   # batches per core
P = 128
CCO = H // P         # 4 contraction chunks for o
CCE = Q // P         # 2 contraction chunks for ee/eo
N0 = 512             # PSUM-bank split of the free dim

# packed output row regions: d_ee | d_eo | d_o | p | q | v
R_DEE, R_DEO, R_DO = 0, NEE, NEE + NEO
R_P, R_Q, R_V = R_DO + NU, R_DO + NU + NEE, R_DO + NU + NEE + NEO
R_TOT = R_V + NU     # 1844

DTYPE_NAME = os.environ.get("KERNEL_DTYPE", "bf16")
if DTYPE_NAME == "bf16":
    import ml_dtypes
    MM_DTYPE = mybir.dt.bfloat16
    NP_IN = ml_dtypes.bfloat16
    OUT_DTYPE = mybir.dt.bfloat16
else:
    MM_DTYPE = mybir.dt.float32r
    NP_IN = np.float32
    OUT_DTYPE = mybir.dt.float32


def _chunks(n, p=P):
    return [(i * p, min(p, n - i * p)) for i in range((n + p - 1) // p)]


EE_CHUNKS = _chunks(NEE)   # [(0,128),(128,103)]
EO_CHUNKS = _chunks(NEO)   # [(0,128),(128,102)]
O_CHUNKS = _chunks(NU)     # 4 chunks


def _dct_matrix(N: int) -> np.ndarray:
    """Orthonormal DCT-II matrix [N, N] in float64."""
    n = np.arange(N, dtype=np.float64)
    C = np.cos(np.pi * (2.0 * n[None, :] + 1.0) * n[:, None] / (2.0 * N))
    s = np.full(N, np.sqrt(2.0 / N))
    s[0] = np.sqrt(1.0 / N)
    return s[:, None] * C


def _build_weights():
    C_T = _dct_matrix(T)
    C_k = _dct_matrix(K)
    W2 = (C_k.T @ C_T[:K, :]).T            # [T, K]
    W2r = W2[::-1, :]
    S = ((W2[:H, :] + W2r[:H, :]) / 2.0)[:, :NU]   # [512, 461]
    A = ((W2[:H, :] - W2r[:H, :]) / 2.0)[:, :NU]   # [512, 461]
    Sp = (S[:Q, :] + S[:Q, ::-1])[:, :NEE]         # [256, 231]
    Sm = (S[:Q, :] - S[:Q, ::-1])[:, :NEO]         # [256, 230]
    wee = np.concatenate([C_T[0:K:4, :Q].T, Sp], axis=1)   # [256, 462]
    weo = np.concatenate([C_T[2:K:4, :Q].T, Sm], axis=1)   # [256, 460]
    wo = np.concatenate([C_T[1:K:2, :H].T, A], axis=1)     # [512, 922]
    return wee.astype(NP_IN), weo.astype(NP_IN), wo.astype(NP_IN)


def _build_bass(loop_repeat: int = 1):
    """loop_repeat>1 wraps the program in a hardware For_i loop (same
    outputs each trip) - used by test.py for slope-based HW timing."""
    f32 = mybir.dt.float32
    nc = bacc.Bacc("TRN2", target_bir_lowering=False, debug=False,
                   num_devices=N_CORES)
    ee_in = nc.dram_tensor("ee", [BPC, Q, D], MM_DTYPE,
                           kind="ExternalInput").ap()
    eo_in = nc.dram_tensor("eo", [BPC, Q, D], MM_DTYPE,
                           kind="ExternalInput").ap()
    o_in = nc.dram_tensor("o", [BPC, H, D], MM_DTYPE,
                          kind="ExternalInput").ap()
    wee_in = nc.dram_tensor("wee", [Q, NEE + NEE], MM_DTYPE,
                            kind="ExternalInput").ap()
    weo_in = nc.dram_tensor("weo", [Q, NEO + NEO], MM_DTYPE,
                            kind="ExternalInput").ap()
    wo_in = nc.dram_tensor("wo", [H, NU + NU], MM_DTYPE,
                           kind="ExternalInput").ap()
    out = nc.dram_tensor("out", [BPC, R_TOT, D], OUT_DTYPE,
                         kind="ExternalOutput").ap()

    ee_r = ee_in.rearrange("b (c p) d -> b p c d", p=P)
    eo_r = eo_in.rearrange("b (c p) d -> b p c d", p=P)
    o_r = o_in.rearrange("b (c p) d -> b p c d", p=P)
    wee_r = wee_in.rearrange("(c p) j -> p c j", p=P)
    weo_r = weo_in.rearrange("(c p) j -> p c j", p=P)
    wo_r = wo_in.rearrange("(c p) j -> p c j", p=P)

    with tile.TileContext(nc) as tc:
        with (
            tc.tile_pool(name="wp", bufs=1) as wp,
            tc.tile_pool(name="xp", bufs=3) as xp,
            tc.tile_pool(name="op", bufs=8) as op,
            tc.tile_pool(name="pp", bufs=4, space="PSUM") as pp,
        ):
            weet = wp.tile([P, CCE, NEE + NEE], MM_DTYPE)
            weot = wp.tile([P, CCE, NEO + NEO], MM_DTYPE)
            wot = wp.tile([P, CCO, NU + NU], MM_DTYPE)
            # weights stream on the ACT HWDGE ring in consumption order
            for (c0, sz) in EE_CHUNKS:
                nc.scalar.dma_start(weet[:, :, c0:c0 + sz],
                                    wee_r[:, :, c0:c0 + sz])
            for (c0, sz) in EO_CHUNKS:
                nc.scalar.dma_start(weot[:, :, c0:c0 + sz],
                                    weo_r[:, :, c0:c0 + sz])
            for (c0, sz) in O_CHUNKS:
                nc.scalar.dma_start(wot[:, :, c0:c0 + sz],
                                    wo_r[:, :, c0:c0 + sz])
            for (c0, sz) in EE_CHUNKS:
                nc.scalar.dma_start(weet[:, :, NEE + c0:NEE + c0 + sz],
                                    wee_r[:, :, NEE + c0:NEE + c0 + sz])
            for (c0, sz) in EO_CHUNKS:
                nc.scalar.dma_start(weot[:, :, NEO + c0:NEO + c0 + sz],
                                    weo_r[:, :, NEO + c0:NEO + c0 + sz])
            for (c0, sz) in O_CHUNKS:
                nc.scalar.dma_start(wot[:, :, NU + c0:NU + c0 + sz],
                                    wo_r[:, :, NU + c0:NU + c0 + sz])

            def mm_group(pt, wtile, ncc, c0, rhs, sz):
                # the 512/256 halves share one weight tile; the dedup pass
                # below collapses their Ldweights into one.
                for cc in range(ncc):
                    st, sp = (cc == 0), (cc == ncc - 1)
                    nc.tensor.matmul(
                        pt[:sz, 0:N0], wtile[:, cc, c0:c0 + sz],
                        rhs[:, cc, 0:N0], start=st, stop=sp)
                    nc.tensor.matmul(
                        pt[:sz, N0:D], wtile[:, cc, c0:c0 + sz],
                        rhs[:, cc, N0:D], start=st, stop=sp)

            def body():
                for b in range(BPC):
                    eet = xp.tile([P, CCE, D], MM_DTYPE, tag="eet")
                    eot = xp.tile([P, CCE, D], MM_DTYPE, tag="eot")
                    ot = xp.tile([P, CCO, D], MM_DTYPE, tag="ot")
                    nc.sync.dma_start(eet[:], ee_r[b])
                    nc.sync.dma_start(eot[:], eo_r[b])
                    nc.sync.dma_start(ot[:], o_r[b])

                    units = []
                    for (r0, sz) in EE_CHUNKS:     # dct rows 4i
                        units.append((weet, CCE, r0, eet, R_DEE + r0, sz))
                    for (r0, sz) in EO_CHUNKS:     # dct rows 4i+2
                        units.append((weot, CCE, r0, eot, R_DEO + r0, sz))
                    for (r0, sz) in O_CHUNKS:      # dct odd rows
                        units.append((wot, CCO, r0, ot, R_DO + r0, sz))
                    for (r0, sz) in EE_CHUNKS:     # p
                        units.append((weet, CCE, NEE + r0, eet, R_P + r0, sz))
                    for (r0, sz) in EO_CHUNKS:     # q
                        units.append((weot, CCE, NEO + r0, eot, R_Q + r0, sz))
                    for (r0, sz) in O_CHUNKS:      # v
                        units.append((wot, CCO, NU + r0, ot, R_V + r0, sz))

                    for i, (wtile, ncc, c0, rhs, dr0, sz) in enumerate(units):
                        pt = pp.tile([P, D], f32, tag="pt")
                        mm_group(pt, wtile, ncc, c0, rhs, sz)
                        so = op.tile([P, D], OUT_DTYPE, tag="so")
                        if i % 2 == 0:
                            nc.vector.tensor_copy(so[:sz, :], pt[:sz, :])
                        else:
                            nc.scalar.copy(so[:sz, :], pt[:sz, :])
                        nc.sync.dma_start(out[b, dr0:dr0 + sz, :], so[:sz, :])

            if loop_repeat > 1:
                with tc.For_i(0, loop_repeat, 1):
                    body()
            else:
                body()
    _dedup_ldweights(nc)
    nc.compile()
    return nc


def _dedup_ldweights(nc):
    """Drop an InstLdweights whose weights AP is identical to the previous
    Ldweights on the PE queue with only Matmult/EventSemaphore between -
    the array already holds those weights. The tile legalizer emits one
    Ldweights per matmul unconditionally; with the batch-pair loop 4
    matmuls share each weight tile, so this removes 3/4 of the (~106 ns
    each) PE-sequencer weight-load slots."""
    pe = mybir.EngineType.PE
    for blk in nc.m.functions[0].blocks:
        insts = list(blk.instructions)
        keep = []
        removed = False
        last_key = None
        for inst in insts:
            if getattr(inst, "engine", None) == pe:
                n = type(inst).__name__
                if n == "InstLdweights":
                    key = str(inst.ins[0])
                    if key == last_key:
                        removed = True
                        continue
                    last_key = key
                elif n not in ("InstMatmult", "InstEventSemaphore"):
                    last_key = None
            keep.append(inst)
        if removed:
            blk.instructions = keep


_CACHE = {}


def _get():
    if "nc" not in _CACHE:
        _CACHE["nc"] = _build_bass()
        _CACHE["w"] = _build_weights()
    return _CACHE["nc"], _CACHE["w"]


def _make_in_maps(x: np.ndarray):
    _, w = _get()
    wee, weo, wo = w
    x = np.ascontiguousarray(x, dtype=np.float32)
    lo = x[:, :H, :]
    hi = x[:, :H - 1:-1, :]
    e = lo + hi
    o = np.ascontiguousarray(lo - hi, dtype=NP_IN)
    ee = np.ascontiguousarray(e[:, :Q, :] + e[:, :Q - 1:-1, :], dtype=NP_IN)
    eo = np.ascontiguousarray(e[:, :Q, :] - e[:, :Q - 1:-1, :], dtype=NP_IN)
    return [
        {"ee": ee[c * BPC:(c + 1) * BPC], "eo": eo[c * BPC:(c + 1) * BPC],
         "o": o[c * BPC:(c + 1) * BPC], "wee": wee, "weo": weo, "wo": wo}
        for c in range(N_CORES)
    ]


def kernel(x: np.ndarray, _results_out=None):
    """x [64, 1024, 768] fp32 -> (x_rec [64, 922, 768], x_dct [64, 922, 768])."""
    nc, _ = _get()
    in_maps = _make_in_maps(x)
    res = run_bass_kernel_spmd(nc, in_maps, core_ids=list(range(N_CORES)))
    if _results_out is not None:
        _results_out.append(res)
    out = np.concatenate([r["out"] for r in res.results], axis=0)
    out = out.astype(np.float32)

    d_ee = out[:, R_DEE:R_DEE + NEE]
    d_eo = out[:, R_DEO:R_DEO + NEO]
    d_o = out[:, R_DO:R_DO + NU]
    pp_ = out[:, R_P:R_P + NEE]
    qq = out[:, R_Q:R_Q + NEO]
    vv = out[:, R_V:R_V + NU]

    x_dct = np.empty((B, K, D), dtype=np.float32)
    x_dct[:, 0::4] = d_ee
    x_dct[:, 2::4] = d_eo
    x_dct[:, 1::2] = d_o

    u = np.empty((B, NU, D), dtype=np.float32)
    u[:, :NEO] = (pp_[:, :NEO] + qq) * 0.5
    u[:, NEO] = pp_[:, NEO] * 0.5
    u[:, NEE:] = ((pp_[:, :NEO] - qq) * 0.5)[:, ::-1]
    x_rec = np.concatenate([u + vv, (u - vv)[:, ::-1, :]], axis=1)
    return np.ascontiguousarray(x_rec), np.ascontiguousarray(x_dct)
